# revision 41
# baseline (speedup 1.0000x reference)
"""AdditiveAttention (Bahdanau) TRN2 Bass kernel, mask-sparse.

softmax(mask ? tanh(vW + MU) @ v : -inf)  over rows, for
B=32, R=4096, D=1024, data-parallel over batch across 8 NeuronCores.

Sparsity: masked rows contribute exactly-0 probabilities, so the kernel
only computes scores for unmasked rows.  The host compacts each batch's
unmasked rows (index gather), the device computes per-partition v-dot
partials for the compacted rows, and the host finishes: partition sum ->
masked softmax -> scatter-back (all O(B R), tiny next to the O(B R D^2)
device matmul).

Batches are load-balanced into 4 per-core "slots" by descending row
count (rank r -> slot r//8, core r%8), so each slot's compiled width is
the max over just 8 batches instead of all 32 — and since the row axis
is the matmul's *moving* axis, widths need no 128-alignment at all.

Device-work minimization: only the big matmul chain runs on device:
  - matrix rows are compacted, cast to fp16 AND pre-transposed to
    [d, r] layout on host, so the PE never runs transpose instructions
    and the DMA is a straight contiguous copy;
  - proj_v = vec @ W (tiny) is computed on host in fp32 and shipped as
    the per-partition tanh bias, so W never reaches the device;
  - U is pre-cast fp16 and pre-arranged into the [p, c, e] SBUF layout;
  - the v-dot partition reduction, exp and normalization are host-side,
    so the device ships the fp16 [128, r] accumulator per block and has
    no reduce matmuls, masks, exp or scaling at all.

Per core (4 slots), per row block (<=1024 columns):
  - 8 e-chunk matmul groups (8 fp16 matmuls x <=2 PSUM-bank halves)
    -> PSUM fp32; tanh+bias on ScalarE -> fp16 inter;
  - v-dot: DVE per-chunk multiply (tensor_scalar) + add (tensor_tensor)
    into an fp16 acc (all-fp16 for the 2x DVE mode); DMA acc out.

A short burst of dummy warmup matmuls keeps the PE busy through the
DVFS p-state ramp while the first real DMAs land.
"""

from contextlib import ExitStack

import numpy as np

import bass_rust
import concourse.bass as bass
import concourse.tile as tile
from concourse import mybir
from concourse import bass_utils

F32 = mybir.dt.float32
F16 = mybir.dt.float16

B, R, D = 32, 4096, 1024
NCORES = 8
BPC = B // NCORES          # batches per core (= slots)
NC_ = D // 128             # d (and e) chunks

_uid = [0]


def _legalize_waits(nc):
    """This walrus accepts at most 1 sync wait per instruction (2 for
    EventSemaphore); Tile's kernel-tail drain piles all terminal waits onto
    one Drain. Split the excess into wait-only EventSemaphores."""
    for f in nc.m.functions:
        for bb in f.blocks:
            insts = list(bb.instructions)
            new_insts = []
            changed = False
            for inst in insts:
                si = inst.sync_info
                waits = list(si.on_wait) if si is not None else []
                cap = 2 if isinstance(inst, mybir.InstEventSemaphore) else 1
                if len(waits) > cap:
                    changed = True
                    keep, rest = waits[:cap], waits[cap:]
                    for i in range(0, len(rest), 2):
                        _uid[0] += 1
                        ev = mybir.InstEventSemaphore(
                            name=f"lw_{inst.name}_{_uid[0]}", ins=[], outs=[]
                        )
                        ev.engine = inst.engine
                        ev.sync_info = bass_rust.SyncInfo(
                            on_wait=list(rest[i : i + 2]), on_update=[]
                        )
                        new_insts.append(ev)
                    inst.sync_info = bass_rust.SyncInfo(
                        on_wait=keep, on_update=list(si.on_update)
                    )
                new_insts.append(inst)
            if changed:
                bb.instructions = new_insts
    return nc


def _chunks(width, step):
    """[(offset, size), ...] covering [0, width) in steps of `step`."""
    return [(o, min(step, width - o)) for o in range(0, width, step)]


N_WARMUP = 12  # dummy 128-col matmuls riding out the PE p-state ramp


def _wide_blocks(S):
    """As many 1024-wide blocks as possible; a remainder <= 128 is merged
    into the final block (js pieces stay single-bank, and a small piece's
    LDWEIGHTS hides under the 512-wide streams around it); a larger
    remainder becomes its own block."""
    q, rem = divmod(S, 1024)
    if rem == 0:
        blocks = [1024] * q
    elif rem <= 128 and q >= 1:
        blocks = [1024] * (q - 1) + [1024 + rem]
    else:
        blocks = [1024] * q + [rem]
    out, o = [], 0
    for w in blocks:
        out.append((o, w)); o += w
    return out


def _slot_blocks(S, last_slot):
    if last_slot and S > 1024 + 352:
        # short final block: the end-of-kernel drain (tanh -> v-dot ->
        # DMA -> barriers) runs on a narrow tile
        tail = 352
        return _wide_blocks(S - tail) + [(S - tail, tail)]
    return _wide_blocks(S)


def _emit(nc, Ss):
    mat_in = [nc.dram_tensor(f"mat{b}", [NC_, 128, Ss[b]], F16,
                             kind="ExternalInput").ap() for b in range(BPC)]
    out = [nc.dram_tensor(f"out{b}", [128, Ss[b]], F16,
                          kind="ExternalOutput").ap() for b in range(BPC)]
    tail_w = (352 if Ss[BPC - 1] > 1024 + 352 else 0)
    out_tail = (nc.dram_tensor("outt", [NC_, 128, tail_w], F16,
                               kind="ExternalOutput").ap() if tail_w else None)
    pv_in = nc.dram_tensor("pv", [128, NC_, BPC], F32,
                           kind="ExternalInput").ap()
    u_in = nc.dram_tensor("u", [128, NC_, NC_, 128], F16,
                          kind="ExternalInput").ap()
    v_in = nc.dram_tensor("v", [128, NC_], F32, kind="ExternalInput").ap()

    slot_blocks = [_slot_blocks(Ss[b], b == BPC - 1) for b in range(BPC)]

    with tile.TileContext(nc) as tc, ExitStack() as ctx:
        consts = ctx.enter_context(tc.tile_pool(name="consts", bufs=1))
        matT_p = ctx.enter_context(tc.tile_pool(name="matT", bufs=3))
        inter_p = ctx.enter_context(tc.tile_pool(name="inter", bufs=3))
        acc_p = ctx.enter_context(tc.tile_pool(name="acc", bufs=2))
        wk_p = ctx.enter_context(tc.tile_pool(name="wk", bufs=2))
        pm_ps = ctx.enter_context(tc.tile_pool(name="pm_ps", bufs=6, space="PSUM"))
        wm_ps = ctx.enter_context(tc.tile_pool(name="wm_ps", bufs=1, space="PSUM"))

        # ---- PE warmup: dummy matmuls with no DMA dependency keep the
        # array busy through the DVFS ramp while the first inputs land.
        scratch = consts.tile([128, 128], F16, tag="scratch")
        nc.vector.memset(scratch[:], 1.0)
        wm = wm_ps.tile([128, 128], F32, tag="wm")
        for w in range(N_WARMUP):
            nc.tensor.matmul(wm[:], scratch[:], scratch[:],
                             start=True, stop=True)

        # ---- sync ring, in arrival-priority order: U chunk 0 (first pm
        # group), tiny consts, then U chunks 1-7.  U is chunk-major
        # [p, k, c, e] so each chunk DMA moves 2KB-contiguous runs on both
        # the HBM and SBUF side (the old e-sliced layout degraded to 256B
        # segments at ~1/3 the ring rate, starving the first block's
        # k-loop of weights).
        u16 = consts.tile([128, NC_, NC_, 128], F16, tag="u16")
        nc.gpsimd.dma_start(u16[:, 0, :, :], u_in[:, 0, :, :])
        v16 = consts.tile([128, NC_], F32, tag="v16")
        nc.sync.dma_start(v16[:], v_in[:])
        pv_sb = consts.tile([128, NC_, BPC], F32, tag="pv")
        nc.sync.dma_start(pv_sb[:], pv_in[:])

        # ---- matrix stream rides the swdge (gpsimd) ring; emission order
        # is arrival order.  The first block lands in a few pieces ordered
        # as the k=0 matmul group consumes them, so the PE can start early
        # without flooding the queue with DMA-trigger instructions.
        loaded = set()
        matT_t = {}

        def ensure_load(b, rb):
            if b >= BPC or rb >= len(slot_blocks[b]):
                return
            if (b, rb) in loaded:
                return
            loaded.add((b, rb))
            r0, rblk = slot_blocks[b][rb]
            matT = matT_p.tile([128, NC_, rblk], F16, tag="matT",
                               name=f"matT_{b}_{rb}")
            matT_t[(b, rb)] = matT
            if b == 0 and rb == 0:
                # last-consumed c-pairs ride the (otherwise idle) sync
                # ring: ~0.5MB less prerequisite cargo on the fast ring.
                # Safe only now that U rides the fast ring itself.
                hw0 = min(512, rblk)
                for c in range(0, NC_, 2):
                    eng = nc.gpsimd if c < NC_ - 2 else nc.sync
                    eng.dma_start(
                        matT[:, c : c + 2, 0:hw0],
                        mat_in[b][c : c + 2, :, r0 : r0 + hw0].rearrange(
                            "c p r -> p c r"))
                if rblk > hw0:
                    nc.gpsimd.dma_start(
                        matT[:, :, hw0:rblk],
                        mat_in[b][:, :, r0 + hw0 : r0 + rblk].rearrange(
                            "c p r -> p c r"))
            else:
                nc.gpsimd.dma_start(
                    matT[:], mat_in[b][:, :, r0 : r0 + rblk].rearrange(
                        "c p r -> p c r"))

        ensure_load(0, 0)

        for k in range(1, NC_):
            nc.gpsimd.dma_start(u16[:, k, :, :], u_in[:, k, :, :])

        ensure_load(0, 1)

        for b in range(BPC):
            for rb, (r0, rblk) in enumerate(slot_blocks[b]):
                sfx = f"{b}_{rb}"
                js = _chunks(rblk, 512)       # PSUM column chunks
                first = b == 0 and rb == 0

                ensure_load(b, rb)
                matT = matT_t[(b, rb)]
                tail = (tail_w and b == BPC - 1
                        and rb == len(slot_blocks[b]) - 1)
                acc = None if tail else acc_p.tile(
                    [128, rblk], F16, tag="acc", name=f"acc_{sfx}")

                for k in range(NC_):
                    # one PSUM tile per (k, j) piece: pool slots are
                    # bank-aligned, so any piece width <= 512 is a legal
                    # single-bank matmul accumulation target
                    pms = [pm_ps.tile([128, jw], F32, tag="pm",
                                      name=f"pm_{sfx}_{k}_{j0}")
                           for j0, jw in js]
                    if first:
                        # j-outer, c-inner: matches the piecewise DMA order
                        for j, (j0, jw) in enumerate(js):
                            for c in range(NC_):
                                nc.tensor.matmul(
                                    pms[j][:],
                                    u16[:, k, c, :],
                                    matT[:, c, j0 : j0 + jw],
                                    start=(c == 0),
                                    stop=(c == NC_ - 1),
                                )
                    else:
                        for c in range(NC_):
                            for j, (j0, jw) in enumerate(js):
                                nc.tensor.matmul(
                                    pms[j][:],
                                    u16[:, k, c, :],
                                    matT[:, c, j0 : j0 + jw],
                                    start=(c == 0),
                                    stop=(c == NC_ - 1),
                                )
                    if k == 0:
                        # prefetch the next block's matrix DMA
                        if rb + 1 < len(slot_blocks[b]):
                            ensure_load(b, rb + 1)
                        else:
                            ensure_load(b + 1, 0)
                            ensure_load(b + 1, 1)
                    for j, (j0, jw) in enumerate(js):
                        inter = inter_p.tile([128, jw], F16, tag="inter",
                                             name=f"inter_{sfx}_{k}_{j0}")
                        nc.scalar.activation(
                            inter[:], pms[j][:],
                            mybir.ActivationFunctionType.Tanh,
                            bias=pv_sb[:, k, b : b + 1], scale=1.0,
                        )
                        if tail:
                            # end-of-kernel block: ship tanh output per
                            # e-chunk as it appears; host does its v-dot
                            nc.gpsimd.dma_start(
                                out_tail[k, :, j0 : j0 + jw], inter[:])
                            continue
                        # v-dot accumulation on DVE: wk = inter * v_k (TS),
                        # acc += wk (TT); all-fp16 for the 2x DVE mode.
                        if k == 0:
                            nc.vector.tensor_scalar_mul(
                                acc[:, j0 : j0 + jw], inter[:], v16[:, 0:1])
                        else:
                            wk = wk_p.tile([128, jw], F16, tag="wk",
                                           name=f"wk_{sfx}_{k}_{j0}")
                            nc.vector.tensor_scalar_mul(
                                wk[:], inter[:], v16[:, k : k + 1])
                            nc.vector.tensor_add(
                                acc[:, j0 : j0 + jw], acc[:, j0 : j0 + jw],
                                wk[:])

                if not tail:
                    nc.sync.dma_start(out[b][:, r0 : r0 + rblk], acc[:])

    return nc


_NC_CACHE = {}


def _get_nc(Ss):
    if Ss not in _NC_CACHE:
        nc = bass.Bass("TRN2", target_bir_lowering=False, debug=False)
        _emit(nc, Ss)
        _legalize_waits(nc)
        _NC_CACHE[Ss] = nc
    return _NC_CACHE[Ss]


def make_plan(matrix_mask):
    """Compacted row indices per batch, slot assignment, slot widths.

    Batches sorted by descending (padded) row count; rank r -> slot r//8,
    core r%8.  Slot width = max row count in the rank group, padded to a
    multiple of 16 (DMA-friendly; no 128-alignment needed since rows ride
    the matmul's moving axis).
    """
    m = np.asarray(matrix_mask) != 0
    idxs = [np.nonzero(m[b])[0] for b in range(m.shape[0])]
    ns = np.array([max(len(i), 1) for i in idxs])
    order = np.argsort(-ns, kind="stable")
    assign = {}          # (core, slot) -> batch
    Ss = []
    for j in range(BPC):
        grp = order[8 * j : 8 * (j + 1)]
        Ss.append(int(-(-max(ns[g] for g in grp) // 8) * 8))
        for i, g in enumerate(grp):
            assign[(i, j)] = int(g)
    return idxs, assign, tuple(Ss)


def make_in_maps(vector, matrix, matrix_mask, w_matrix, u_matrix, v_vector,
                 idxs, assign, Ss):
    vector = np.ascontiguousarray(vector, dtype=np.float32)
    matrix = np.asarray(matrix)
    w = np.asarray(w_matrix, dtype=np.float32)
    # proj_v on host (tiny; fp32, more accurate than the device fp16 path)
    pvh = vector @ w                                    # (B, D)
    # U pre-cast fp16 into the SBUF layout [p, c, e]
    u = np.ascontiguousarray(
        np.asarray(u_matrix, dtype=np.float32).astype(np.float16)
        .reshape(NC_, 128, NC_, 128).transpose(1, 2, 0, 3))
    v16 = np.ascontiguousarray(
        np.asarray(v_vector, dtype=np.float32).reshape(NC_, 128).T)
    in_maps = []
    for c in range(NCORES):
        im = {"u": u, "v": v16}
        pv_c = np.empty((BPC, D), dtype=np.float32)
        for j in range(BPC):
            gb = assign[(c, j)]
            S = Ss[j]
            idx = idxs[gb]
            n = len(idx)
            pad = np.zeros(S - n, dtype=np.intp) if n == 0 else \
                np.full(S - n, idx[0], dtype=np.intp)
            idx_pad = np.concatenate([idx.astype(np.intp), pad])
            # gather + cast + transpose to [d, r], then [c, p, r]
            g16 = matrix[gb][idx_pad].astype(np.float16)      # (S, D)
            im[f"mat{j}"] = np.ascontiguousarray(g16.T).reshape(NC_, 128, S)
            pv_c[j] = pvh[gb]
        im["pv"] = np.ascontiguousarray(
            pv_c.T.reshape(NC_, 128, BPC).transpose(1, 0, 2),
            dtype=np.float32)
        in_maps.append(im)
    return in_maps


def scatter_out(results, idxs, assign, Ss, v16):
    """Host-side partition sum + masked softmax + scatter-back."""
    out = np.zeros((B, R), dtype=np.float32)
    S3 = Ss[BPC - 1]
    tail_w = (352 if S3 > 1024 + 352 else 0)
    for c in range(NCORES):
        for j in range(BPC):
            gb = assign[(c, j)]
            idx = idxs[gb]
            if len(idx) == 0:
                out[gb, :] = 1.0 / R   # softmax of all-equal (-1e9) logits
                continue
            dev = results[c][f"out{j}"]          # (128, S) fp16
            s = dev.astype(np.float32).sum(axis=0)
            if tail_w and j == BPC - 1:
                # final block shipped as raw tanh chunks; v-dot here
                t = results[c]["outt"].astype(np.float32)  # (NC_, 128, tw)
                s[S3 - tail_w :] = np.einsum("kpr,pk->r", t, v16)
            s = s[: len(idx)]
            e = np.exp(s - s.max())
            out[gb, idx] = e / e.sum()
    return out


def kernel(vector, matrix, matrix_mask, w_matrix, u_matrix, v_vector):
    idxs, assign, Ss = make_plan(matrix_mask)
    nc = _get_nc(Ss)
    in_maps = make_in_maps(vector, matrix, matrix_mask, w_matrix, u_matrix,
                           v_vector, idxs, assign, Ss)
    res = bass_utils.run_bass_kernel_spmd(nc, in_maps, core_ids=list(range(NCORES)))
    return scatter_out(res.results, idxs, assign, Ss, in_maps[0]["v"])
